# revision 11
# baseline (speedup 1.0000x reference)
"""Paged GQA decode attention (sparse_attention) on 8 TRN2 NeuronCores.

Sharding: tensor-parallel by KV head (8 heads -> 8 cores). Each core gets its
head's slice of the KV pool as a single interleaved bf16 tensor kv_il[t] =
[k_hi(128) | v_hi(128)], so ONE 512 B dma_gather descriptor per token fetches
both K and V (the cost-model DMA floor: >=512 B descriptors run at full bus
bandwidth; two half-size gathers would cost 2x).

Per core dataflow (specialized at build time on the actual seq_lens split,
identical across cores):
  gather: kv[128, slots, 256] bf16 natural layout (tokens on partitions)
  K^T:    per 128-token slot, PE transpose (identity matmul) -> PSUM, then
          PSUM->SBUF copy alternating between ACT and DVE engines
  QK:     scores^T[s,4] = kT @ qhi + kT @ qlo  (q split hi|lo bf16 on host,
          SCALE prefolded; K single bf16)
  exp:    one ACT Exp per group -> p bf16 in SBUF; pad tails zeroed by
          per-section mask columns (tensor_scalar per-partition multiply)
  PV:     o accum [4, j*128..]: p_slot @ v_slot (single bf16 matmul per slot)
  sums:   ones^T @ p -> per-slot partial sums; final reduction and softmax
          normalization on host (elementwise on the tiny [B,HQ,D] output).
"""

import numpy as np
import ml_dtypes

import concourse.bacc as bacc
import concourse.bass as bass
import concourse.mybir as mybir
import concourse.tile as tile
from concourse.bass_utils import run_bass_kernel_spmd
from concourse.masks import make_identity

B, S, HQ, HKV, D, G = 32, 2048, 32, 8, 128, 4
POOL = B * S
HALF = POOL // 2
SCALE = D ** -0.5
NCORES = 8
GROUPS = 8
RPG = B // GROUPS  # requests per group

BF16 = ml_dtypes.bfloat16

_prog_cache: dict = {}
LAST_RESULT = None  # test.py introspection


def _pad128(n):
    return (n + 127) // 128 * 128


def _layout(meta):
    """meta[g][h][j] = valid token count of request j in half h of group g.

    Returns per group: per-half padded counts/slot tables, per-request slot
    lists + sum ranges + mask column ids, and each group-half's column offset
    into the merged idx tensor.
    """
    info = []
    mask_cols = []  # list of (g, h, j, valid_in_last_slot) -> mask col id
    icol = 0  # running column offset into the merged idx tensor
    for g in range(GROUPS):
        lo_secs, hi_secs = meta[g]
        halves = []
        for h, secs in enumerate((lo_secs, hi_secs)):
            starts, slot_cnt = [], []
            pos = 0
            for j in range(RPG):
                starts.append(pos // 128)
                slot_cnt.append(_pad128(secs[j]) // 128)
                pos += _pad128(secs[j])
            halves.append(dict(n=pos, slots=pos // 128, ioff=icol,
                               starts=starts, slot_cnt=slot_cnt, secs=secs))
            icol += pos // 16
        n_lo_slots = halves[0]["slots"]
        nslots = n_lo_slots + halves[1]["slots"]
        req_slots, req_ranges, req_masks = [], [], []
        for j in range(RPG):
            slots, ranges, masks = [], [], []
            for h in (0, 1):
                hh = halves[h]
                base = 0 if h == 0 else n_lo_slots
                s0, cnt = hh["starts"][j], hh["slot_cnt"][j]
                if cnt:
                    ranges.append((base + s0, cnt))
                    for li in range(cnt):
                        slots.append((h, s0 + li, base + s0 + li))
                    tail = hh["secs"][j] % 128
                    if tail:  # partial last slot -> needs mask col
                        mid = len(mask_cols)
                        mask_cols.append((g, h, j, tail))
                        masks.append((base + s0 + cnt - 1, mid))
            req_slots.append(slots)
            req_ranges.append(ranges)
            req_masks.append(masks)
        info.append(dict(halves=halves, nslots=nslots,
                         req_slots=req_slots, req_ranges=req_ranges,
                         req_masks=req_masks))
    return info, mask_cols, icol


def _build_program(meta):
    info, mask_cols, idx_w = _layout(meta)
    n_mask = max(1, len(mask_cols))
    dt = mybir.dt
    nc = bacc.Bacc(trn_type="TRN2")

    kv_il = nc.dram_tensor("kv_il", [POOL, 256], dt.bfloat16, kind="ExternalInput")
    qhiT = nc.dram_tensor("qhiT", [128, 128], dt.bfloat16, kind="ExternalInput")
    qloT = nc.dram_tensor("qloT", [128, 128], dt.bfloat16, kind="ExternalInput")
    maskc_d = nc.dram_tensor("maskc", [128, n_mask], dt.float32, kind="ExternalInput")
    idx_w = max(1, idx_w)
    idx_d = nc.dram_tensor("idx_all", [128, idx_w], dt.int16, kind="ExternalInput")
    OC = RPG * D  # output cols per group
    o_dram = nc.dram_tensor("o_un", [G, B * D], dt.float32, kind="ExternalOutput")
    s_dram = nc.dram_tensor("sums", [GROUPS, 512], dt.float32, kind="ExternalOutput")

    CS = 16  # chunk size in slots (2048 tokens per gather)

    with tile.TileContext(nc) as tc:
        with (
            tc.tile_pool(name="const", bufs=1) as cpool,
            tc.tile_pool(name="idx", bufs=8) as idxp,
            tc.tile_pool(name="kv", bufs=10) as kvp,
            tc.tile_pool(name="kt", bufs=12) as ktp,
            tc.tile_pool(name="pt", bufs=3) as ptp,
            tc.tile_pool(name="stg", bufs=2) as stgp,
            tc.tile_pool(name="ps_tr", bufs=3, space="PSUM") as pstr,
            tc.tile_pool(name="ps_sc", bufs=3, space="PSUM") as pssc,
            tc.tile_pool(name="ps_pv", bufs=2, space="PSUM") as pspv,
        ):
            qhi_t = cpool.tile([128, 128], dt.bfloat16, tag="qhi")
            qlo_t = cpool.tile([128, 128], dt.bfloat16, tag="qlo")
            ones_t = cpool.tile([128, 1], dt.bfloat16, tag="ones")
            ident_t = cpool.tile([128, 128], dt.bfloat16, tag="ident")
            mask_t = cpool.tile([128, n_mask], dt.float32, tag="maskc")
            nc.sync.dma_start(out=qhi_t[:], in_=qhiT[:])
            nc.sync.dma_start(out=qlo_t[:], in_=qloT[:])
            nc.sync.dma_start(out=mask_t[:], in_=maskc_d[:])
            nc.vector.memset(ones_t[:], 1.0)
            make_identity(nc, ident_t[:])

            ncopy = 0  # global copy counter for ACT/DVE alternation
            for g in range(GROUPS):
                gi = info[g]
                nslots = gi["nslots"]
                ncols = 4 * nslots
                if nslots == 0:
                    z = stgp.tile([33, OC], dt.float32, tag="ostg")
                    nc.vector.memset(z[:], 0.0)
                    nc.sync.dma_start(out=o_dram[0:G, OC * g:OC * (g + 1)],
                                      in_=z[0:G, :])
                    continue
                n_lo_slots = gi["halves"][0]["slots"]

                def owner(h, loc):
                    hh = gi["halves"][h]
                    return max(jj for jj in range(RPG)
                               if hh["starts"][jj] <= loc)

                # mask application keyed by global slot of each section tail
                mask_by_slot = {}
                for j in range(RPG):
                    for (gslot, mid) in gi["req_masks"][j]:
                        mask_by_slot[gslot] = mid

                pt = ptp.tile([128, ncols], dt.bfloat16, tag="pt")
                kv_tiles = {}  # (h, chunk) -> tile
                # --- chunked gather -> transpose/copy -> QK -> exp ---------
                for h in (0, 1):
                    hh = gi["halves"][h]
                    slots_h = hh["slots"]
                    if slots_h == 0:
                        continue
                    gbase = 0 if h == 0 else n_lo_slots
                    for ci, c0 in enumerate(range(0, slots_h, CS)):
                        cs = min(CS, slots_h - c0)
                        n = 128 * cs
                        # idx slice for this chunk (8 idx cols per slot)
                        ic0 = hh["ioff"] + 8 * c0
                        it = idxp.tile([128, 8 * cs], dt.int16, tag="idx")
                        nc.sync.dma_start(out=it[:],
                                          in_=idx_d[:, ic0:ic0 + 8 * cs])
                        src = (kv_il[0:HALF, :] if h == 0
                               else kv_il[HALF:POOL, :])
                        kv = kvp.tile([128, cs, 256], dt.bfloat16, tag="kv")
                        nc.gpsimd.dma_gather(
                            out_ap=kv[:], in_ap=src, idxs_ap=it[:],
                            num_idxs=n, num_idxs_reg=n, elem_size=256,
                            transpose=False, single_packet=False)
                        kv_tiles[(h, ci)] = kv

                        # K^T: transpose 4 slots/PSUM tile + one batched copy
                        sc = pssc.tile([128, 4 * cs], dt.float32, tag="sc")
                        for i0 in range(0, cs, 4):
                            nb = min(4, cs - i0)
                            tp = pstr.tile([128, 512], dt.bfloat16, tag="tr")
                            kt = ktp.tile([128, 512], dt.bfloat16, tag="kt")
                            for i in range(nb):
                                nc.tensor.transpose(
                                    tp[:, 128 * i:128 * (i + 1)],
                                    kv[:, i0 + i, 0:128], ident_t[:])
                            w = 128 * nb
                            if ncopy & 1:
                                nc.vector.tensor_copy(out=kt[:, 0:w],
                                                      in_=tp[:, 0:w])
                            else:
                                nc.scalar.activation(
                                    kt[:, 0:w], tp[:, 0:w],
                                    mybir.ActivationFunctionType.Copy)
                            ncopy += 1
                            # QK for these slots
                            for i in range(nb):
                                loc = c0 + i0 + i
                                b = RPG * g + owner(h, loc)
                                out = sc[:, 4 * (i0 + i):4 * (i0 + i) + 4]
                                ksl = kt[:, 128 * i:128 * (i + 1)]
                                nc.tensor.matmul(out, ksl,
                                                 qhi_t[:, 4 * b:4 * b + 4],
                                                 start=True, stop=False)
                                nc.tensor.matmul(out, ksl,
                                                 qlo_t[:, 4 * b:4 * b + 4],
                                                 start=False, stop=True)
                        # exp for the chunk + tail masks
                        pc0 = 4 * (gbase + c0)
                        nc.scalar.activation(pt[:, pc0:pc0 + 4 * cs], sc[:],
                                             mybir.ActivationFunctionType.Exp)
                        for i in range(cs):
                            gs = gbase + c0 + i
                            if gs in mask_by_slot:
                                mid = mask_by_slot[gs]
                                cc = 4 * gs
                                nc.vector.tensor_scalar_mul(
                                    out=pt[:, cc:cc + 4],
                                    in0=pt[:, cc:cc + 4],
                                    scalar1=mask_t[:, mid:mid + 1])

                # --- PV + sums ---------------------------------------------
                # pv rows 0..3 = PV accum; row 32 = softmax partial sums
                pvs = pspv.tile([33, OC], dt.float32, tag="pv")
                pv = pvs[0:G, :]
                sm = pvs[32:33, :]
                for j in range(RPG):
                    slots = gi["req_slots"][j]
                    oc = 128 * j
                    if not slots:
                        nc.vector.memset(pv[0:G, oc:oc + 128], 0.0)
                        continue
                    last = len(slots) - 1
                    for si, (h, loc, gslot) in enumerate(slots):
                        kvt = kv_tiles[(h, loc // CS)]
                        nc.tensor.matmul(
                            pv[0:G, oc:oc + 128],
                            pt[:, 4 * gslot:4 * gslot + 4],
                            kvt[:, loc % CS, 128:256],
                            start=(si == 0), stop=(si == last))
                    for (s0, cnt) in gi["req_ranges"][j]:
                        nc.tensor.matmul(
                            sm[0:1, 4 * s0:4 * (s0 + cnt)], ones_t[:, 0:1],
                            pt[:, 4 * s0:4 * (s0 + cnt)], start=True, stop=True)

                ostg = stgp.tile([33, OC], dt.float32, tag="ostg")
                nc.vector.tensor_copy(out=ostg[0:G, :], in_=pvs[0:G, :])
                nc.scalar.activation(ostg[32:33, 0:ncols],
                                     pvs[32:33, 0:ncols],
                                     mybir.ActivationFunctionType.Copy)
                nc.sync.dma_start(out=o_dram[0:G, OC * g:OC * (g + 1)],
                                  in_=ostg[0:G, :])
                nc.sync.dma_start(out=s_dram[g:g + 1, 0:ncols],
                                  in_=ostg[32:33, 0:ncols])

    nc.compile()
    return nc, info, mask_cols


def prepare(inputs):
    q = np.asarray(inputs["q"], np.float32)
    k = np.asarray(inputs["k"], np.float32)
    v = np.asarray(inputs["v"], np.float32)
    k_buffer = np.asarray(inputs["k_buffer"], np.float32)
    v_buffer = np.asarray(inputs["v_buffer"], np.float32)
    req_to_token = np.asarray(inputs["req_to_token"])
    req_pool_indices = np.asarray(inputs["req_pool_indices"])
    seq_lens = np.asarray(inputs["seq_lens"]).astype(np.int64)
    out_cache_loc = np.asarray(inputs["out_cache_loc"]).astype(np.int64)

    # store_kv_cache scatter (tiny: 32 rows) + per-request token lists
    kb = k_buffer.copy()
    vb = v_buffer.copy()
    kb[out_cache_loc] = k.reshape(B, HKV, D)
    vb[out_cache_loc] = v.reshape(B, HKV, D)
    tok = req_to_token[req_pool_indices]

    # smallest group first (fast pipeline fill), next-smallest last (short
    # drain tail), the rest biggest-first in between
    asc = list(np.argsort(seq_lens, kind="stable"))
    head, tail_, mid = asc[RPG:2 * RPG], asc[:RPG], asc[2 * RPG:][::-1]
    order = np.array(head + mid + tail_, dtype=np.int64)

    meta = []
    idx_blocks = []
    for g in range(GROUPS):
        lo_secs, hi_secs = [], []
        for h in (0, 1):
            parts = []
            secs = lo_secs if h == 0 else hi_secs
            for j in range(RPG):
                b = int(order[RPG * g + j])
                t = tok[b, :seq_lens[b]].astype(np.int64)
                tl = t[t < HALF] if h == 0 else t[t >= HALF] - HALF
                secs.append(len(tl))
                arr = np.zeros(_pad128(len(tl)), np.int64)
                arr[:len(tl)] = tl
                parts.append(arr)
            full = np.concatenate(parts)
            if len(full):
                # [16, n/16] wrap, replicated into all 8 GPSIMD-core stripes
                idx_blocks.append(
                    np.tile(full.astype(np.int16).reshape(-1, 16).T, (8, 1)))
        meta.append((tuple(lo_secs), tuple(hi_secs)))
    meta = tuple(meta)
    if idx_blocks:
        idx_all = np.ascontiguousarray(np.concatenate(idx_blocks, axis=1))
    else:
        idx_all = np.zeros((128, 1), np.int16)

    if meta not in _prog_cache:
        _prog_cache[meta] = _build_program(meta)
    nc, info, mask_cols = _prog_cache[meta]

    maskc = np.ones((128, max(1, len(mask_cols))), np.float32)
    for mid, (_, _, _, tail) in enumerate(mask_cols):
        maskc[:, mid] = (np.arange(128) < tail).astype(np.float32)

    in_maps = []
    for c in range(NCORES):
        k_hi = kb[:, c, :].astype(BF16)
        v_hi = vb[:, c, :].astype(BF16)
        qc = (q.reshape(B, HKV, G, D)[order, c] * SCALE).reshape(B * G, D)
        qT = np.ascontiguousarray(qc.T)
        q_hi = qT.astype(BF16)
        q_lo = (qT - q_hi.astype(np.float32)).astype(BF16)
        im = {
            "kv_il": np.ascontiguousarray(np.concatenate([k_hi, v_hi], axis=1)),
            "qhiT": np.ascontiguousarray(q_hi),
            "qloT": np.ascontiguousarray(q_lo),
            "maskc": maskc,
            "idx_all": idx_all,
        }
        in_maps.append(im)
    return nc, info, in_maps, order


def postprocess(results, info, order, cores=None):
    OC = RPG * D
    out = np.zeros((B, HQ, D), np.float32)
    for c in (cores if cores is not None else range(NCORES)):
        o_un = results[c]["o_un"]
        sums = results[c]["sums"]
        for g in range(GROUPS):
            gi = info[g]
            for j in range(RPG):
                b = int(order[RPG * g + j])
                stot = np.zeros(G, np.float64)
                for (s0, cnt) in gi["req_ranges"][j]:
                    seg = sums[g, 4 * s0:4 * (s0 + cnt)].astype(np.float64)
                    stot += seg.reshape(cnt, G).sum(axis=0)
                ov = o_un[:, OC * g + 128 * j:OC * g + 128 * (j + 1)]
                with np.errstate(divide="ignore", invalid="ignore"):
                    out[b, c * G:(c + 1) * G, :] = ov / stot[:, None]
    return out.reshape(B, HQ * D).astype(np.float32)


def kernel(**inputs):
    global LAST_RESULT
    nc, info, in_maps, order = prepare(inputs)
    res = run_bass_kernel_spmd(nc, in_maps, core_ids=list(range(NCORES)),
                               trace=False)
    LAST_RESULT = res
    return postprocess(res.results, info, order)
